# revision 7
# baseline (speedup 1.0000x reference)
"""Trainium2 8-core kernel for a BailingMoE decoder layer.

Sharding:
  - Tensor-parallel attention: 2 q-heads (+ GQA kv-head) per core.
  - Token-parallel norms/router on T/8 shards, stitched with collectives.
  - Expert-parallel MoE: 4 experts/core, on-device top-4 routing with
    capacity padding. Dispatch tables are built with one-hot matmuls
    (token-id + combine-weight payload against rank one-hots), tokens are
    fetched with natural-mode dma_gather + on-chip transposes, and the
    expert outputs ride dma_scatter_add back into the accumulator.
  - Shared-expert MLP tensor-parallel over SI; its partial and the
    routed partials ride one final ReduceScatter.

Matmuls run in bf16 (fp32 PSUM accumulation).  The attention output
ReduceScatter and the router logits stay fp32 so the top-4 choices
track the fp32 reference closely; the logits are all-gathered (tiny)
so every core ranks tokens identically.
"""

import numpy as np

import concourse.bacc as bacc
import concourse.bass as bass
import concourse.mybir as mybir
import concourse.tile as tile
from concourse.bass import IndirectOffsetOnAxis
from concourse.bass_utils import run_bass_kernel_spmd

T, HID = 2048, 2048
H, KV, D = 16, 4, 128
E, K, MI, SI = 32, 4, 1408, 2816
EPS = 1e-6
THETA = 1e6

NC = 8
TSH = T // NC        # 256
HPC = H // NC        # 2
EPC = E // NC        # 4
SIS = SI // NC       # 352
CAP = 384
NEG = -1.0e30

F32 = mybir.dt.float32
BF16 = mybir.dt.bfloat16
U32 = mybir.dt.uint32
I16 = mybir.dt.int16

AF = mybir.ActivationFunctionType
OP = mybir.AluOpType
AX = mybir.AxisListType

KT = HID // 128      # 16
NT = T // 128        # 16
NG = T // 512        # 4
MIT = MI // 128      # 11
CAPT = CAP // 128    # 3
HALF = D // 2


def _bf(x):
    import ml_dtypes
    return np.ascontiguousarray(np.asarray(x), dtype=None).astype(
        ml_dtypes.bfloat16)


def build_nc():
    nc = bacc.Bacc("TRN2", target_bir_lowering=False, debug=False,
                   num_devices=NC, num_swdge_queues=4)
    rg = [list(range(NC))]

    def inp(name, shape, dt=BF16):
        return nc.dram_tensor(name, list(shape), dt, kind="ExternalInput")

    io = dict(
        xT_bf=inp("xT_bf", (HID, T)),
        x_rows=inp("x_rows", (TSH, HID), F32),
        x_rows_tok=inp("x_rows_tok", (TSH, HID), F32),
        wqkv=inp("wqkv", (HID, 4 * D)),
        qnw=inp("qnw", (D, 1), F32),
        knw=inp("knw", (D, 1), F32),
        cosT=inp("cosT", (D, T), F32),
        sinT=inp("sinT", (D, T), F32),
        wo_r=inp("wo_r", (HPC * D, HID)),
        router=inp("router", (HID, E), F32),
        wsg=inp("wsg", (HID, SIS)),
        wsu=inp("wsu", (HID, SIS)),
        wsd=inp("wsd", (SIS, HID)),
        weg=inp("weg", (EPC, HID, MI)),
        weu=inp("weu", (EPC, HID, MI)),
        wed=inp("wed", (EPC, MI, HID)),
        ident=inp("ident", (128, 128)),
        identF=inp("identF", (128, 128), F32),
        trilS=inp("trilS", (128, 128), F32),
        ones128=inp("ones128", (128, 128), F32),
        onescol=inp("onescol", (128, 1), F32),
        onesrow=inp("onesrow", (1, 128), F32),
        iota32=inp("iota32", (128, E), F32),
        mcapbig=inp("mcapbig", (128, EPC), F32),
        iotaCE=inp("iotaCE", (128, EPC, CAP), F32),
        idcol=inp("idcol", (128, NT), F32),
        tril01=inp("tril01", (128, 128)),
        onescolb=inp("onescolb", (128, 1)),
        out_sh=nc.dram_tensor("out", [TSH, HID], F32, kind="ExternalOutput"),
        s_sh=nc.dram_tensor("s_sh", [TSH, 1], F32),
        s_all=nc.dram_tensor("s_all", [T, 1], F32, addr_space="Shared"),
        wo_part=nc.dram_tensor("wo_part", [T, HID], BF16),
        rs1=nc.dram_tensor("rs1", [TSH, HID], BF16),
        lg_sh=nc.dram_tensor("lg_sh", [TSH, E], F32),
        lg_all=nc.dram_tensor("lg_all", [T, E], F32, addr_space="Shared"),
        x2n_sh=nc.dram_tensor("x2n_sh", [TSH, HID], BF16),
        x2t_sh=nc.dram_tensor("x2t_sh", [HID, TSH], BF16),
        x2g=nc.dram_tensor("x2g", [T + 16, HID], BF16, addr_space="Shared"),
        x2t_all=nc.dram_tensor("x2t_all", [NC * HID, TSH], BF16,
                               addr_space="Shared"),
        ids_dram=nc.dram_tensor("ids_dram", [EPC * CAP, 1], F32),
        accum=nc.dram_tensor("accum", [T + 16, HID], BF16),
        rsf=nc.dram_tensor("rsf", [TSH, HID], BF16),
    )

    with tile.TileContext(nc) as tc:
        _build(tc, nc, rg, io)
    nc.compile()
    return nc


def _build(tc, nc, rg, io):
    g = lambda k: io[k]

    # =====================================================================
    # Phases 1-3: ln1 scales, QKV, attention, wo partial, ReduceScatter
    # =====================================================================
    with tc.tile_pool(name="const", bufs=1) as cpool:
        ident_sb = cpool.tile([128, 128], BF16, tag="ident")
        nc.sync.dma_start(ident_sb[:], g("ident").ap())
        cos_sb = cpool.tile([D, T], F32, tag="cos")
        nc.sync.dma_start(cos_sb[:], g("cosT").ap())
        sin_sb = cpool.tile([D, T], F32, tag="sin")
        nc.sync.dma_start(sin_sb[:], g("sinT").ap())
        qnw_sb = cpool.tile([D, 1], F32, tag="qnw")
        nc.sync.dma_start(qnw_sb[:], g("qnw").ap())
        knw_sb = cpool.tile([D, 1], F32, tag="knw")
        nc.sync.dma_start(knw_sb[:], g("knw").ap())
        onescol_sb = cpool.tile([128, 1], F32, tag="onescol")
        nc.sync.dma_start(onescol_sb[:], g("onescol").ap())
        onesrow_sb = cpool.tile([1, 128], F32, tag="onesrow")
        nc.sync.dma_start(onesrow_sb[:], g("onesrow").ap())
        tril01_sb = cpool.tile([128, 128], BF16, tag="tril01")
        nc.sync.dma_start(tril01_sb[:], g("tril01").ap())
        onescolb_sb = cpool.tile([128, 1], BF16, tag="onescolb")
        nc.sync.dma_start(onescolb_sb[:], g("onescolb").ap())
        identf_sb = cpool.tile([128, 128], F32, tag="identf")
        nc.sync.dma_start(identf_sb[:], g("identF").ap())
        eps_t = cpool.tile([128, 1], F32, tag="eps")
        nc.vector.memset(eps_t[:], EPS)
        epsD_t = cpool.tile([128, 1], F32, tag="epsD")
        nc.vector.memset(epsD_t[:], float(D) * EPS)

        # --- s = rsqrt(mean(x^2)+eps) on my token shard; AllGather ---
        with tc.tile_pool(name="p1", bufs=2) as p1:
            for i in range(TSH // 128):
                xr = p1.tile([128, HID], F32, tag="xr")
                nc.sync.dma_start(xr[:],
                                  g("x_rows_tok")[i * 128:(i + 1) * 128, :])
                sq = p1.tile([128, HID], F32, tag="sq")
                ssq = p1.tile([128, 1], F32, tag="ssq")
                nc.scalar.activation(sq[:], xr[:], AF.Square,
                                     accum_out=ssq[:])
                sr = p1.tile([128, 1], F32, tag="sr")
                nc.scalar.activation(sr[:], ssq[:], AF.Sqrt,
                                     scale=1.0 / HID, bias=eps_t[:])
                sv = p1.tile([128, 1], F32, tag="sv")
                nc.vector.reciprocal(sv[:], sr[:])
                nc.sync.dma_start(g("s_sh")[i * 128:(i + 1) * 128, :], sv[:])
        nc.gpsimd.collective_compute(
            "AllGather", OP.bypass, replica_groups=rg,
            ins=[g("s_sh").ap().opt()], outs=[g("s_all").ap().opt()])

        # --- QKV into [D, T] layout ---
        with tc.tile_pool(name="qk_f32", bufs=1) as qkp:
            qkT = [qkp.tile([128, T], F32, tag=f"qk{m}", name=f"qkT{m}")
                   for m in range(3)]
            vT = qkp.tile([128, T], F32, tag="vT")

            with tc.tile_pool(name="wqkvp", bufs=1) as wp, \
                 tc.tile_pool(name="xt", bufs=3) as xtp, \
                 tc.tile_pool(name="qkps", bufs=2, space="PSUM") as qkps:
                wq_sb = wp.tile([128, KT, 4 * D], BF16)
                nc.sync.dma_start(
                    wq_sb[:],
                    g("wqkv").ap().rearrange("(k p) m -> p k m", p=128))
                for n in range(NG):
                    ps = [qkps.tile([128, 512], F32, tag=f"ps{m}",
                                     name=f"ps{m}_{n}")
                          for m in range(4)]
                    for k in range(KT):
                        xt = xtp.tile([128, 512], BF16, tag="xt")
                        nc.sync.dma_start(
                            xt[:], g("xT_bf")[k * 128:(k + 1) * 128,
                                              n * 512:(n + 1) * 512])
                        for m in range(4):
                            nc.tensor.matmul(
                                ps[m][:], wq_sb[:, k, m * 128:(m + 1) * 128],
                                xt[:], start=(k == 0), stop=(k == KT - 1))
                    for m in range(3):
                        nc.vector.tensor_copy(
                            qkT[m][:, n * 512:(n + 1) * 512], ps[m][:])
                    nc.vector.tensor_copy(vT[:, n * 512:(n + 1) * 512],
                                          ps[3][:])

            # --- v -> [T, D] natural, scaled by s, bf16 ---
            s_t = qkp.tile([128, NT], F32, tag="s_t")
            nc.sync.dma_start(
                s_t[:], g("s_all").ap().rearrange("(s p) o -> p (s o)",
                                                  p=128))
            v_nat = qkp.tile([128, NT, D], BF16, tag="v_nat")
            with tc.tile_pool(name="vtp", bufs=4, space="PSUM") as vtp:
                for j in range(NT):
                    vv = vtp.tile([128, 128], F32, tag="vv")
                    nc.tensor.transpose(vv[:], vT[:, j * 128:(j + 1) * 128],
                                        identf_sb[:])
                    nc.vector.tensor_scalar_mul(v_nat[:, j, :], vv[:],
                                                s_t[:, j:j + 1])

            # --- qk-norm scales ---
            rqk = [qkp.tile([1, T], F32, tag=f"rqk{m}", name=f"rqk{m}")
                   for m in range(3)]
            with tc.tile_pool(name="nrm", bufs=1) as nrm, \
                 tc.tile_pool(name="nps", bufs=1, space="PSUM") as nps, \
                 tc.tile_pool(name="bcps", bufs=1, space="PSUM") as bcps:
                for m in range(3):
                    sq = nrm.tile([128, T], F32, tag="nsq")
                    nc.vector.tensor_mul(sq[:], qkT[m][:], qkT[m][:])
                    pss = nps.tile([1, T], F32, tag="pss")
                    for n in range(NG):
                        nc.tensor.matmul(
                            pss[:, n * 512:(n + 1) * 512], onescol_sb[:],
                            sq[:, n * 512:(n + 1) * 512],
                            start=True, stop=True)
                    srt = nrm.tile([1, T], F32, tag="srt")
                    if m < 2:
                        # q: 1/sqrt(ssq + D*eps) = D^-0.5 * rsqrt(mean+eps)
                        # (the softmax D^-0.5 rides along)
                        nc.scalar.activation(srt[:], pss[:], AF.Sqrt,
                                             bias=epsD_t[0:1, :])
                    else:
                        # k: plain rsqrt(mean+eps)
                        nc.scalar.activation(srt[:], pss[:], AF.Sqrt,
                                             scale=1.0 / D,
                                             bias=eps_t[0:1, :])
                    nc.vector.reciprocal(rqk[m][:], srt[:])
                # k: rms-scale broadcast via K=1 matmul; qnorm weight on q
                bc = bcps.tile([128, T], F32, tag="bc")
                for n in range(NG):
                    nc.tensor.matmul(
                        bc[:, n * 512:(n + 1) * 512], onesrow_sb[:],
                        rqk[2][:, n * 512:(n + 1) * 512],
                        start=True, stop=True)
                kn = nrm.tile([128, T], F32, tag="kn")
                nc.vector.scalar_tensor_tensor(
                    kn[:], qkT[2][:], knw_sb[:], bc[:],
                    op0=OP.mult, op1=OP.mult)
                qn = []
                for m in range(2):
                    qq = nrm.tile([128, T], F32, tag=f"qn{m}")
                    nc.vector.tensor_scalar_mul(qq[:], qkT[m][:], qnw_sb[:])
                    qn.append(qq)

                # rope -> bf16:  out = q*[cos;cos] + swap(q)*[-sin;sin]
                qk_bf = []
                srcs = [qn[0], qn[1], kn]
                for m in range(3):
                    ob = qkp.tile([128, T], BF16, tag=f"rope{m}")
                    qs = nrm.tile([128, T], F32, tag="qs")
                    nc.scalar.copy(qs[0:HALF, :], srcs[m][HALF:D, :])
                    nc.scalar.copy(qs[HALF:D, :], srcs[m][0:HALF, :])
                    tt1 = nrm.tile([128, T], F32, tag="tt1")
                    tt2 = nrm.tile([128, T], F32, tag="tt2")
                    nc.vector.tensor_mul(tt1[:], srcs[m][:], cos_sb[:])
                    nc.vector.tensor_mul(tt2[:], qs[:], sin_sb[:])
                    nc.vector.tensor_add(ob[:], tt1[:], tt2[:])
                    qk_bf.append(ob)

            # ------------- attention (2 heads, causal GQA) ---------------
            # Transposed-score formulation: per kv-tile j the scores land
            # as [kv, t] so exp output feeds p@v directly (no per-block
            # transpose matmuls).  Softmax runs without max subtraction
            # (qk-normed scores are bounded by sqrt(D)); the denominator is
            # accumulated with a ones-column matmul and divided out at the
            # end of each 512-token chunk.  wo + a chunked bf16
            # ReduceScatter ride behind the attention compute.
            attnT = [qkp.tile([128, T], BF16, tag=f"attnT{h}",
                                name=f"attnT{h}")
                     for h in range(HPC)]
            wo_sb = qkp.tile([128, HPC, HID], BF16, tag="wo_sb")
            nc.sync.dma_start(
                wo_sb[:],
                g("wo_r").ap().rearrange("(h p) m -> p h m", p=128))
            with tc.tile_pool(name="srow", bufs=2, space="PSUM") as srow, \
                 tc.tile_pool(name="aden", bufs=1, space="PSUM") as aden, \
                 tc.tile_pool(name="apat", bufs=1, space="PSUM") as apat, \
                 tc.tile_pool(name="awop", bufs=2, space="PSUM") as awop, \
                 tc.tile_pool(name="att", bufs=3) as att, \
                 tc.tile_pool(name="awsb", bufs=3) as awsb:
                for n in range(NG):
                    den_t = aden.tile([64, 512], F32, tag="den")
                    pats = [apat.tile([128, 512], F32, tag=f"pat{h}",
                                      name=f"pat{h}_{n}")
                            for h in range(HPC)]
                    qsc = []
                    for h in range(HPC):
                        bcq = awop.tile([128, 512], F32, tag="po")
                        nc.tensor.matmul(
                            bcq[:], onesrow_sb[:],
                            rqk[h][:, n * 512:(n + 1) * 512],
                            start=True, stop=True)
                        qs = att.tile([128, 512], BF16, tag=f"qsc{h}")
                        nc.vector.tensor_mul(
                            qs[:], qk_bf[h][:, n * 512:(n + 1) * 512],
                            bcq[:])
                        qsc.append(qs)
                    jn = 4 * (n + 1)
                    for j in range(jn):
                        c0 = max(0, j * 128 - 512 * n)
                        for h in range(HPC):
                            srw = srow.tile([128, 512], F32, tag="srw")
                            nc.tensor.matmul(
                                srw[:, c0:512],
                                qk_bf[2][:, j * 128:(j + 1) * 128],
                                qsc[h][:, c0:512],
                                start=True, stop=True)
                            pT = att.tile([128, 512], BF16, tag=f"pT{h}")
                            nc.scalar.activation(pT[:, c0:512],
                                                 srw[:, c0:512], AF.Exp)
                            if j >= 4 * n:
                                nc.vector.tensor_mul(
                                    pT[:, c0:c0 + 128], pT[:, c0:c0 + 128],
                                    tril01_sb[:])
                            # NB: the PSUM pending-zero region of start=True
                            # covers only the OUT AP's partitions, so each
                            # head's 1-partition den row needs its own start.
                            nc.tensor.matmul(
                                den_t[h * 32:h * 32 + 1, c0:512],
                                onescolb_sb[:], pT[:, c0:512],
                                start=(j == 0),
                                stop=(j == jn - 1),
                                skip_group_check=True)
                            nc.tensor.matmul(
                                pats[h][:, c0:512], v_nat[:, j, :],
                                pT[:, c0:512],
                                start=(j == 0), stop=(j == jn - 1),
                                skip_group_check=True)
                    for h in range(HPC):
                        rden = att.tile([1, 512], F32, tag=f"rden{h}")
                        nc.vector.reciprocal(rden[:],
                                             den_t[h * 32:h * 32 + 1, :])
                        bcd = awop.tile([128, 512], F32, tag="po")
                        nc.tensor.matmul(bcd[:], onesrow_sb[:], rden[:],
                                         start=True, stop=True)
                        bcs = att.tile([128, 512], F32, tag="bcs")
                        nc.vector.tensor_copy(bcs[:], bcd[:])
                        nc.vector.tensor_mul(
                            attnT[h][:, n * 512:(n + 1) * 512],
                            pats[h][:], bcs[:])
                    # wo for this 512-token chunk, then its ReduceScatter
                    for tt in range(4):
                        t0 = (n * 4 + tt) * 128
                        for nn in range(4):
                            po = awop.tile([128, 512], F32, tag="po")
                            for h in range(HPC):
                                nc.tensor.matmul(
                                    po[:], attnT[h][:, t0:t0 + 128],
                                    wo_sb[:, h, nn * 512:(nn + 1) * 512],
                                    start=(h == 0), stop=(h == HPC - 1))
                            ob = awsb.tile([128, 512], BF16, tag="ob")
                            nc.vector.tensor_copy(ob[:], po[:])
                            nc.sync.dma_start(
                                g("wo_part")[t0:t0 + 128,
                                             nn * 512:(nn + 1) * 512], ob[:])
                    nc.gpsimd.collective_compute(
                        "ReduceScatter", OP.add, replica_groups=rg,
                        ins=[g("wo_part")[n * 512:(n + 1) * 512, :].opt()],
                        outs=[g("rs1")[n * 64:(n + 1) * 64, :].opt()])

    # =====================================================================
    # Phase 4: residual2, ln2, x2 (f32 + bf16), logits; AGs
    # =====================================================================
    with tc.tile_pool(name="keep", bufs=1) as keep:
        resid2 = keep.tile([128, TSH // 128, HID], F32, tag="resid2")
        ident2 = keep.tile([128, 128], F32, tag="ident2")
        nc.sync.dma_start(ident2[:], g("identF").ap())
        identb = keep.tile([128, 128], BF16, tag="identb")
        nc.sync.dma_start(identb[:], g("ident").ap())
        eps4_t = keep.tile([128, 1], F32, tag="eps4")
        nc.vector.memset(eps4_t[:], EPS)

        with tc.tile_pool(name="p4", bufs=2) as p4, \
             tc.tile_pool(name="p4f", bufs=1) as p4f, \
             tc.tile_pool(name="p4ps", bufs=4, space="PSUM") as p4ps, \
             tc.tile_pool(name="lgps", bufs=2, space="PSUM") as lgps:
            xt2f = p4f.tile([128, KT, TSH], F32, tag="xt2f")
            router_sb = p4f.tile([128, KT, E], F32, tag="router")
            nc.sync.dma_start(
                router_sb[:],
                g("router").ap().rearrange("(k p) e -> p k e", p=128))
            for i in range(TSH // 128):
                rsb = p4.tile([128, HID], BF16, tag="rsb")
                nc.sync.dma_start(rsb[:], g("rs1")[i * 128:(i + 1) * 128, :])
                xr = p4.tile([128, HID], F32, tag="xr4")
                nc.sync.dma_start(xr[:], g("x_rows")[i * 128:(i + 1) * 128, :])
                nc.vector.tensor_add(resid2[:, i, :], rsb[:], xr[:])
                sq = p4.tile([128, HID], F32, tag="sq4")
                ssq = p4.tile([128, 1], F32, tag="ssq4")
                nc.scalar.activation(sq[:], resid2[:, i, :], AF.Square,
                                     accum_out=ssq[:])
                sr = p4.tile([128, 1], F32, tag="sr4")
                nc.scalar.activation(sr[:], ssq[:], AF.Sqrt, scale=1.0 / HID,
                                     bias=eps4_t[:])
                sv = p4.tile([128, 1], F32, tag="sv4")
                nc.vector.reciprocal(sv[:], sr[:])
                x2f = p4.tile([128, HID], F32, tag="x2f")
                nc.vector.tensor_scalar_mul(x2f[:], resid2[:, i, :], sv[:])
                x2b = p4.tile([128, HID], BF16, tag="x2b")
                nc.vector.tensor_copy(x2b[:], x2f[:])
                nc.sync.dma_start(g("x2n_sh")[i * 128:(i + 1) * 128, :],
                                  x2b[:])
                for kh in range(KT):
                    pt = p4ps.tile([128, 128], F32, tag="pt4")
                    nc.tensor.transpose(
                        pt[:], x2f[:, kh * 128:(kh + 1) * 128], ident2[:])
                    nc.vector.tensor_copy(
                        xt2f[:, kh, i * 128:(i + 1) * 128], pt[:])
                    tb = p4.tile([128, 128], BF16, tag="tb4")
                    nc.vector.tensor_copy(
                        tb[:], xt2f[:, kh, i * 128:(i + 1) * 128])
                    nc.sync.dma_start(
                        g("x2t_sh")[kh * 128:(kh + 1) * 128,
                                    i * 128:(i + 1) * 128], tb[:])
                # fp32 logits for this token tile
                lg = lgps.tile([128, E], F32, tag="lg")
                for kh in range(KT):
                    nc.tensor.matmul(
                        lg[:], xt2f[:, kh, i * 128:(i + 1) * 128],
                        router_sb[:, kh, :],
                        start=(kh == 0), stop=(kh == KT - 1))
                lgo = p4.tile([128, E], F32, tag="lgo")
                nc.vector.tensor_copy(lgo[:], lg[:])
                nc.sync.dma_start(g("lg_sh")[i * 128:(i + 1) * 128, :],
                                  lgo[:])

            nc.gpsimd.collective_compute(
                "AllGather", OP.bypass, replica_groups=rg,
                ins=[g("lg_sh").ap().opt()], outs=[g("lg_all").ap().opt()])
            nc.gpsimd.collective_compute(
                "AllGather", OP.bypass, replica_groups=rg,
                ins=[g("x2t_sh").ap().opt()], outs=[g("x2t_all").ap().opt()])
            nc.gpsimd.collective_compute(
                "AllGather", OP.bypass, replica_groups=rg,
                ins=[g("x2n_sh").ap().opt()],
                outs=[g("x2g")[0:T, :].opt()])
            zz = p4.tile([16, HID], BF16, tag="zz")
            nc.vector.memset(zz[:], 0.0)
            nc.sync.dma_start(g("x2g")[T:T + 16, :], zz[:])

        # =================================================================
        # Phase 5: routing + dispatch tables (one-hot matmuls, no DGE)
        # =================================================================
        with tc.tile_pool(name="rtc", bufs=1) as rtc:
            iota_sb = rtc.tile([128, E], F32, tag="iota")
            nc.sync.dma_start(iota_sb[:], g("iota32").ap())
            mcap_sb = rtc.tile([128, EPC], F32, tag="mcap")
            nc.sync.dma_start(mcap_sb[:], g("mcapbig").ap())
            iotace_sb = rtc.tile([128, EPC, CAP], F32, tag="iotace")
            nc.sync.dma_start(iotace_sb[:], g("iotaCE").ap())
            idcol_sb = rtc.tile([128, NT], F32, tag="idcol")
            nc.sync.dma_start(idcol_sb[:], g("idcol").ap())
            tril_sb = rtc.tile([128, 128], F32, tag="tril")
            nc.sync.dma_start(tril_sb[:], g("trilS").ap())
            ones_sb = rtc.tile([128, 128], F32, tag="ones")
            nc.sync.dma_start(ones_sb[:], g("ones128").ap())

            w4_all = rtc.tile([128, NT, EPC], F32, tag="w4")
            msk = rtc.tile([128, NT, E], F32, tag="msk")
            pay = rtc.tile([128, NT, 1 + EPC], F32, tag="pay")
            idx16 = rtc.tile([128, EPC, CAP // 16], I16, tag="idx16")
            wsl_sb = rtc.tile([128, EPC, CAPT], F32, tag="wsl")
            ids_sb = rtc.tile([128, EPC, CAPT], F32, tag="idsb")

            with tc.tile_pool(name="rt", bufs=3) as rt, \
                 tc.tile_pool(name="rkps", bufs=2, space="PSUM") as rkps, \
                 tc.tile_pool(name="tabp", bufs=1, space="PSUM") as tabp:
                tabps = tabp.tile([128, EPC, CAPT, 1 + EPC], F32, tag="tab")
                for i in range(NT):
                    lgt = rt.tile([128, E], F32, tag="lgt")
                    nc.sync.dma_start(lgt[:],
                                      g("lg_all")[i * 128:(i + 1) * 128, :])
                    gs = rt.tile([128, E], F32, tag="gs")
                    nc.scalar.activation(gs[:], lgt[:], AF.Sigmoid)
                    mx8 = rt.tile([128, 8], F32, tag="mx8")
                    ix8 = rt.tile([128, 8], U32, tag="ix8")
                    nc.vector.max_with_indices(mx8[:], ix8[:], gs[:])
                    sm = rt.tile([128, 1], F32, tag="sm")
                    nc.vector.tensor_reduce(sm[:], mx8[:, 0:K], axis=AX.X,
                                            op=OP.add)
                    rsm = rt.tile([128, 1], F32, tag="rsm")
                    nc.vector.reciprocal(rsm[:], sm[:])
                    nc.vector.tensor_scalar_mul(w4_all[:, i, :], mx8[:, 0:K],
                                                rsm[:])
                    ixf = rt.tile([128, K], F32, tag="ixf")
                    nc.vector.tensor_copy(ixf[:], ix8[:, 0:K])
                    comb = rt.tile([128, E], F32, tag="comb")
                    nc.vector.tensor_scalar(
                        comb[:], iota_sb[:], ixf[:, 0:1], w4_all[:, i, 0:1],
                        op0=OP.is_equal, op1=OP.mult)
                    oh = rt.tile([128, E], F32, tag="ohc")
                    for j in range(1, K):
                        nc.vector.tensor_scalar(
                            oh[:], iota_sb[:], ixf[:, j:j + 1],
                            w4_all[:, i, j:j + 1],
                            op0=OP.is_equal, op1=OP.mult)
                        nc.vector.tensor_add(comb[:], comb[:], oh[:])
                    nc.vector.tensor_scalar(msk[:, i, :], comb[:], 0.0, None,
                                            op0=OP.is_gt)
                    # dispatch payload: [token_id+1, combine weights e0..e3]
                    nc.vector.tensor_copy(pay[:, i, 0:1], idcol_sb[:, i:i + 1])
                    nc.vector.tensor_copy(pay[:, i, 1:1 + EPC],
                                          comb[:, 0:EPC])
                    # rank of token within each local expert (prefix count)
                    rk = rkps.tile([128, E], F32, tag="rk")
                    for s in range(i + 1):
                        nc.tensor.matmul(
                            rk[:], tril_sb[:] if s == i else ones_sb[:],
                            msk[:, s, :], start=(s == 0), stop=(s == i))
                    pen = rt.tile([128, EPC], F32, tag="pen")
                    nc.vector.tensor_scalar(
                        pen[:], msk[:, i, 0:EPC], -1.0e6, None, op0=OP.mult)
                    nc.vector.tensor_add(pen[:], pen[:], mcap_sb[:])
                    ovf = rt.tile([128, EPC], F32, tag="ovf")
                    nc.vector.tensor_scalar(
                        ovf[:], rk[:, 0:EPC], float(CAP) - 0.5, 1.0e6,
                        op0=OP.is_ge, op1=OP.mult)
                    offs = rt.tile([128, EPC], F32, tag="offs")
                    nc.vector.tensor_add(offs[:], pen[:], ovf[:])
                    nc.vector.tensor_add(offs[:], offs[:], rk[:, 0:EPC])
                    # one-hot of slot per expert; accumulate payload table
                    for e in range(EPC):
                        ohs = rt.tile([128, CAP], F32, tag="ohs")
                        nc.vector.tensor_scalar(
                            ohs[:], iotace_sb[:, e, :], offs[:, e:e + 1],
                            None, op0=OP.is_equal)
                        for sc in range(CAPT):
                            # All 12 (e, sc) groups live in one PSUM bank;
                            # start=True pending-zeroes the whole 2KB bank,
                            # so only the very first matmul may set it.
                            nc.tensor.matmul(
                                tabps[:, e, sc, :],
                                ohs[:, sc * 128:(sc + 1) * 128],
                                pay[:, i, :],
                                start=(i == 0 and e == 0 and sc == 0),
                                stop=(i == NT - 1),
                                skip_group_check=True)

                # unpack table: token ids -> DRAM (for idx16); weights -> SBUF
                for e in range(EPC):
                    for sc in range(CAPT):
                        nc.vector.tensor_copy(ids_sb[:, e, sc:sc + 1],
                                              tabps[:, e, sc, 0:1])
                        nc.vector.tensor_copy(wsl_sb[:, e, sc:sc + 1],
                                              tabps[:, e, sc, 1 + e:2 + e])
                        nc.sync.dma_start(
                            g("ids_dram")[e * CAP + sc * 128:
                                          e * CAP + (sc + 1) * 128, :],
                            ids_sb[:, e, sc:sc + 1])
                # decode per-expert wrapped token lists: id = slot - 1,
                # empty slots (0) -> dummy row T
                for e in range(EPC):
                    idf = rt.tile([16, CAP // 16], F32, tag="idf")
                    nc.sync.dma_start(
                        idf[:],
                        g("ids_dram")[e * CAP:(e + 1) * CAP, 0:1].rearrange(
                            "(s p) o -> p (s o)", p=16))
                    idm = rt.tile([16, CAP // 16], F32, tag="idm")
                    nc.vector.tensor_scalar(
                        idm[:], idf[:], 0.5, float(T + 1),
                        op0=OP.is_lt, op1=OP.mult)
                    nc.vector.tensor_add(idm[:], idm[:], idf[:])
                    nc.vector.tensor_scalar(idm[:], idm[:], -1.0, None,
                                            op0=OP.add)
                    idh = rt.tile([16, CAP // 16], I16, tag="idh")
                    nc.vector.tensor_copy(idh[:], idm[:])
                    for r in range(8):
                        nc.sync.dma_start(idx16[16 * r:16 * (r + 1), e, :],
                                          idh[:])

            # =============================================================
            # Phase 6: shared-expert MLP -> accum rows
            # =============================================================
            def x2t_chunk_ap(k, n):
                v = g("x2t_all").ap().rearrange("(c k p) s -> p k c s",
                                                p=128, k=KT)
                return v[:, k, 2 * n:2 * n + 2, :]

            with tc.tile_pool(name="shw", bufs=1) as shw, \
                 tc.tile_pool(name="shx", bufs=18) as shx, \
                 tc.tile_pool(name="shb", bufs=3) as shb, \
                 tc.tile_pool(name="shh", bufs=2) as shh, \
                 tc.tile_pool(name="shps", bufs=2, space="PSUM") as shps, \
                 tc.tile_pool(name="shdps", bufs=1, space="PSUM") as shdps:
                wsg_sb = shw.tile([128, KT, SIS], BF16, tag="wsg")
                nc.sync.dma_start(
                    wsg_sb[:],
                    g("wsg").ap().rearrange("(k p) m -> p k m", p=128))
                wsu_sb = shw.tile([128, KT, SIS], BF16, tag="wsu")
                nc.sync.dma_start(
                    wsu_sb[:],
                    g("wsu").ap().rearrange("(k p) m -> p k m", p=128))
                wsd_sb = shw.tile([128, 3, HID], BF16, tag="wsd")
                mdims = [128, 128, SIS - 256]
                for m in range(3):
                    nc.sync.dma_start(
                        wsd_sb[0:mdims[m], m, :],
                        g("wsd")[m * 128:m * 128 + mdims[m], :])
                for n in range(NG):
                    xts = []
                    for k in range(KT):
                        xt = shx.tile([128, 512], BF16, tag="shxt")
                        nc.sync.dma_start(xt[:], x2t_chunk_ap(k, n))
                        xts.append(xt)
                    hsh = shh.tile([128, 3, 512], BF16, tag="hsh")
                    for m in range(3):
                        md = mdims[m]
                        pg = shps.tile([128, 512], F32, tag="pg")
                        pu = shps.tile([128, 512], F32, tag="pu")
                        for k in range(KT):
                            nc.tensor.matmul(
                                pg[0:md, :],
                                wsg_sb[:, k, m * 128:m * 128 + md],
                                xts[k][:], start=(k == 0),
                                stop=(k == KT - 1))
                            nc.tensor.matmul(
                                pu[0:md, :],
                                wsu_sb[:, k, m * 128:m * 128 + md],
                                xts[k][:], start=(k == 0),
                                stop=(k == KT - 1))
                        sg = shb.tile([128, 512], BF16, tag="sg")
                        nc.scalar.activation(sg[0:md, :], pg[0:md, :],
                                             AF.Silu)
                        nc.vector.tensor_mul(hsh[0:md, m, :], sg[0:md, :],
                                             pu[0:md, :])
                    for ts in range(4):
                        tok0 = n * 512 + ts * 128
                        pd4 = shdps.tile([128, 4, 512], F32, tag="pd4")
                        for m in range(3):
                            md = mdims[m]
                            for nh in range(4):
                                nc.tensor.matmul(
                                    pd4[:, nh, :],
                                    hsh[0:md, m, ts * 128:(ts + 1) * 128],
                                    wsd_sb[0:md, m, nh * 512:(nh + 1) * 512],
                                    start=(m == 0), stop=(m == 2))
                        for nh in range(4):
                            ob = shb.tile([128, 512], BF16, tag="shob")
                            nc.vector.tensor_copy(ob[:], pd4[:, nh, :])
                            nc.sync.dma_start(
                                g("accum")[tok0:tok0 + 128,
                                           nh * 512:(nh + 1) * 512], ob[:])

            # =============================================================
            # Phase 7: routed experts
            # =============================================================
            MIG = [(0, 4), (4, 8), (8, MIT)]      # mi groups for weights
            with tc.tile_pool(name="ew", bufs=18) as ewp, \
                 tc.tile_pool(name="edw", bufs=2) as edwp, \
                 tc.tile_pool(name="exn", bufs=2) as exn, \
                 tc.tile_pool(name="exf", bufs=1) as exf, \
                 tc.tile_pool(name="eh", bufs=2) as ehp, \
                 tc.tile_pool(name="ey", bufs=2) as eyp, \
                 tc.tile_pool(name="esc", bufs=3) as esc, \
                 tc.tile_pool(name="etp", bufs=2, space="PSUM") as etp, \
                 tc.tile_pool(name="eps", bufs=2, space="PSUM") as eps, \
                 tc.tile_pool(name="edps", bufs=1, space="PSUM") as edps:
                for e in range(EPC):
                    xet_nat = exn.tile([128, CAPT, HID], BF16, tag="xetn")
                    nc.gpsimd.dma_gather(
                        out_ap=xet_nat[:], in_ap=g("x2g").ap(),
                        idxs_ap=idx16[:, e, :], num_idxs=CAP,
                        num_idxs_reg=CAP, elem_size=HID, transpose=False,
                        queue_num=e % 2)
                    xet = exf.tile([128, KT, CAP], BF16, tag="xet")
                    for sc in range(CAPT):
                        for k in range(KT):
                            tp = etp.tile([128, 128], BF16, tag="tp")
                            nc.tensor.transpose(
                                tp[:], xet_nat[:, sc, k * 128:(k + 1) * 128],
                                identb[:])
                            nc.vector.tensor_copy(
                                xet[:, k, sc * 128:(sc + 1) * 128], tp[:])
                    hsb = ehp.tile([128, MIT, CAP], BF16, tag="hsb")
                    for (m0, m1) in MIG:
                        mw = (m1 - m0) * 128
                        wgt, wut = [], []
                        for k in range(KT):
                            wg = ewp.tile([128, 512], BF16, tag="wg")
                            nc.sync.dma_start(
                                wg[:, 0:mw],
                                g("weg")[e, k * 128:(k + 1) * 128,
                                         m0 * 128:m1 * 128])
                            wu = ewp.tile([128, 512], BF16, tag="wu")
                            nc.sync.dma_start(
                                wu[:, 0:mw],
                                g("weu")[e, k * 128:(k + 1) * 128,
                                         m0 * 128:m1 * 128])
                            wgt.append(wg)
                            wut.append(wu)
                        for mi in range(m0, m1):
                            mo = (mi - m0) * 128
                            pg = eps.tile([128, CAP], F32, tag="epg")
                            pu = eps.tile([128, CAP], F32, tag="epu")
                            for k in range(KT):
                                nc.tensor.matmul(
                                    pg[:], wgt[k][:, mo:mo + 128],
                                    xet[:, k, :], start=(k == 0),
                                    stop=(k == KT - 1))
                                nc.tensor.matmul(
                                    pu[:], wut[k][:, mo:mo + 128],
                                    xet[:, k, :], start=(k == 0),
                                    stop=(k == KT - 1))
                            sg = esc.tile([128, CAP], BF16, tag="esg")
                            nc.scalar.activation(sg[:], pg[:], AF.Silu)
                            nc.vector.tensor_mul(hsb[:, mi, :], sg[:], pu[:])
                    ysb = eyp.tile([128, CAPT, HID], BF16, tag="ysb")
                    for hh in range(2):
                        wdt = edwp.tile([128, MIT, 1024], BF16, tag="wd")
                        nc.sync.dma_start(
                            wdt[:],
                            g("wed")[e, :, hh * 1024:(hh + 1) * 1024]
                            .rearrange("(m p) n -> p m n", p=128))
                        for ct in range(CAPT):
                            pd = edps.tile([128, 2, 512], F32, tag="epd")
                            for mi in range(MIT):
                                for nh in range(2):
                                    nc.tensor.matmul(
                                        pd[:, nh, :],
                                        hsb[:, mi, ct * 128:(ct + 1) * 128],
                                        wdt[:, mi, nh * 512:(nh + 1) * 512],
                                        start=(mi == 0), stop=(mi == MIT - 1))
                            for nh in range(2):
                                nc.vector.tensor_scalar_mul(
                                    ysb[:, ct, hh * 1024 + nh * 512:
                                        hh * 1024 + (nh + 1) * 512],
                                    pd[:, nh, :], wsl_sb[:, e, ct:ct + 1])
                    nc.gpsimd.dma_scatter_add(
                        out_ap=g("accum").ap(), in_ap=ysb[:],
                        idxs_ap=idx16[:, e, :], num_idxs=CAP,
                        num_idxs_reg=CAP, elem_size=HID,
                        queue_num=2 + (e % 2))

        # =================================================================
        # Phase 8: final ReduceScatter + residual + output
        # =================================================================
        nc.gpsimd.collective_compute(
            "ReduceScatter", OP.add, replica_groups=rg,
            ins=[g("accum")[0:T, :].opt()], outs=[g("rsf").ap().opt()])
        with tc.tile_pool(name="p8", bufs=2) as p8:
            for i in range(TSH // 128):
                rb = p8.tile([128, HID], BF16, tag="rb8")
                nc.sync.dma_start(rb[:], g("rsf")[i * 128:(i + 1) * 128, :])
                ov = p8.tile([128, HID], F32, tag="ov8")
                nc.vector.tensor_add(ov[:], rb[:], resid2[:, i, :])
                nc.sync.dma_start(g("out_sh")[i * 128:(i + 1) * 128, :],
                                  ov[:])


_nc_cache = None
TRACE = False          # set by test.py to capture exec_time_ns
LAST_RESULT = None


def prepare_in_maps_for_sim(inputs):
    """Host-side prep identical to kernel(); returns per-core in_maps."""
    return _prepare_in_maps({k: np.asarray(v) for k, v in inputs.items()})


def _get_nc():
    global _nc_cache
    if _nc_cache is None:
        _nc_cache = build_nc()
    return _nc_cache


def kernel(**inputs):
    inputs = {k: np.asarray(v) for k, v in inputs.items()}
    in_maps = _prepare_in_maps(inputs)
    nc = _get_nc()
    global LAST_RESULT
    res = run_bass_kernel_spmd(nc, in_maps, core_ids=list(range(NC)),
                               trace=TRACE)
    LAST_RESULT = res
    out = np.empty((T, HID), np.float32)
    for c in range(NC):
        out[_core_rows(c)] = res.results[c]["out"]
    return out.astype(np.float32)


def _core_rows(c):
    """Global token rows owned by core c (chunked-RS1 ownership)."""
    return np.concatenate(
        [np.arange(64) + n * 512 + c * 64 for n in range(NG)])


def _prepare_in_maps(inputs):
    pos = inputs["positions"].astype(np.float32)
    hs = inputs["hidden_states"].astype(np.float32)
    ln1_w = inputs["ln1_w"].astype(np.float32)
    ln2_w = inputs["ln2_w"].astype(np.float32)

    inv = 1.0 / (THETA ** (np.arange(HALF, dtype=np.float32) / HALF))
    ang = pos[None, :].astype(np.float64) * inv[:, None].astype(np.float64)
    cos_h = np.cos(ang).astype(np.float32)
    sin_h = np.sin(ang).astype(np.float32)
    cosT = np.vstack([cos_h, cos_h])
    sinT = np.vstack([-sin_h, sin_h])

    xT_bf = _bf(hs.T)
    wq_f = ln1_w[:, None] * inputs["wq"].astype(np.float32)
    wk_f = ln1_w[:, None] * inputs["wk"].astype(np.float32)
    wv_f = ln1_w[:, None] * inputs["wv"].astype(np.float32)
    wo_f = inputs["wo"].astype(np.float32)
    router_f = ln2_w[:, None] * inputs["router_w"].astype(np.float32)
    wsg_f = ln2_w[:, None] * inputs["ws_gate"].astype(np.float32)
    wsu_f = ln2_w[:, None] * inputs["ws_up"].astype(np.float32)
    wsd_f = inputs["ws_down"].astype(np.float32)
    weg_f = ln2_w[None, :, None] * inputs["we_gate"].astype(np.float32)
    weu_f = ln2_w[None, :, None] * inputs["we_up"].astype(np.float32)
    wed_f = inputs["we_down"].astype(np.float32)

    ident = np.eye(128, dtype=np.float32)
    trilS = np.triu(np.ones((128, 128), np.float32), 1)   # [k,m]=1 iff k<m
    ones128 = np.ones((128, 128), np.float32)
    onescol = np.ones((128, 1), np.float32)
    onesrow = np.ones((1, 128), np.float32)
    mcapbig = np.tile(
        (np.arange(EPC, dtype=np.float32) * CAP + 1.0e6)[None, :], (128, 1))
    iotaCE = np.tile(
        (np.arange(EPC, dtype=np.float32)[:, None] * CAP
         + np.arange(CAP, dtype=np.float32)[None, :])[None, :, :],
        (128, 1, 1))
    idcol = (np.arange(NT, dtype=np.float32)[None, :] * 128
             + np.arange(128, dtype=np.float32)[:, None] + 1.0)
    tril01 = (np.arange(128)[:, None] <= np.arange(128)[None, :]).astype(
        np.float32)

    in_maps = []
    for c in range(NC):
        kvh = (HPC * c) * KV // H
        perm = list(range(EPC * c, EPC * (c + 1))) + \
            [e for e in range(E) if not (EPC * c <= e < EPC * (c + 1))]
        iota32 = np.tile(np.asarray(perm, np.float32)[None, :], (128, 1))
        wqkv_c = np.concatenate([
            wq_f[:, (HPC * c) * D:(HPC * c + 2) * D],
            wk_f[:, kvh * D:(kvh + 1) * D],
            wv_f[:, kvh * D:(kvh + 1) * D],
        ], axis=1)
        m = {
            "xT_bf": xT_bf,
            "x_rows": np.ascontiguousarray(hs[_core_rows(c), :]),
            "x_rows_tok": np.ascontiguousarray(hs[c * TSH:(c + 1) * TSH, :]),
            "wqkv": _bf(wqkv_c),
            "qnw": inputs["qnorm_w"].astype(np.float32).reshape(D, 1),
            "knw": inputs["knorm_w"].astype(np.float32).reshape(D, 1),
            "cosT": cosT, "sinT": sinT,
            "wo_r": _bf(wo_f[c * HPC * D:(c + 1) * HPC * D, :]),
            "router": np.ascontiguousarray(router_f),
            "wsg": _bf(wsg_f[:, c * SIS:(c + 1) * SIS]),
            "wsu": _bf(wsu_f[:, c * SIS:(c + 1) * SIS]),
            "wsd": _bf(wsd_f[c * SIS:(c + 1) * SIS, :]),
            "weg": _bf(weg_f[EPC * c:EPC * (c + 1)]),
            "weu": _bf(weu_f[EPC * c:EPC * (c + 1)]),
            "wed": _bf(wed_f[EPC * c:EPC * (c + 1)]),
            "ident": _bf(ident), "identF": ident,
            "trilS": trilS, "ones128": ones128,
            "onescol": onescol, "onesrow": onesrow,
            "iota32": iota32, "mcapbig": mcapbig,
            "iotaCE": iotaCE, "idcol": idcol,
            "tril01": _bf(tril01),
            "onescolb": _bf(onescol),
        }
        in_maps.append(m)
    return in_maps


# revision 18
# speedup vs baseline: 1.2560x; 1.2560x over previous
"""Trainium2 8-core kernel for a BailingMoE decoder layer.

Sharding:
  - Tensor-parallel attention: 2 q-heads (+ GQA kv-head) per core.
  - Token-parallel norms/router on T/8 shards, stitched with collectives.
  - Expert-parallel MoE: 4 experts/core, on-device top-4 routing with
    capacity padding. Dispatch tables are built with one-hot matmuls
    (token-id + combine-weight payload against rank one-hots), tokens are
    fetched with natural-mode dma_gather + on-chip transposes, and the
    expert outputs ride dma_scatter_add back into the accumulator.
  - Shared-expert MLP tensor-parallel over SI; its partial and the
    routed partials ride one final ReduceScatter.

Matmuls run in bf16 (fp32 PSUM accumulation).  The attention output
ReduceScatter and the router logits stay fp32 so the top-4 choices
track the fp32 reference closely; the logits are all-gathered (tiny)
so every core ranks tokens identically.
"""

import numpy as np

import concourse.bacc as bacc
import concourse.bass as bass
import concourse.mybir as mybir
import concourse.tile as tile
from concourse.bass import IndirectOffsetOnAxis
from concourse.bass_utils import run_bass_kernel_spmd

T, HID = 2048, 2048
H, KV, D = 16, 4, 128
E, K, MI, SI = 32, 4, 1408, 2816
EPS = 1e-6
THETA = 1e6

NC = 8
TSH = T // NC        # 256
HPC = H // NC        # 2
EPC = E // NC        # 4
SIS = SI // NC       # 352
CAP = 384
NEG = -1.0e30

F32 = mybir.dt.float32
BF16 = mybir.dt.bfloat16
FP8 = mybir.dt.float8e4
U32 = mybir.dt.uint32
I16 = mybir.dt.int16

DR = mybir.MatmulPerfMode.DoubleRow
WSC = 64.0           # fp8 expert-weight scale
HSC = 8.0            # fp8 h (swiglu activation) scale
MIT2 = 12            # MIT padded to even for DoubleRow pairing

AF = mybir.ActivationFunctionType
OP = mybir.AluOpType
AX = mybir.AxisListType

KT = HID // 128      # 16
NT = T // 128        # 16
NG = T // 512        # 4
MIT = MI // 128      # 11
CAPT = CAP // 128    # 3
HALF = D // 2


def _bf(x):
    import ml_dtypes
    return np.ascontiguousarray(np.asarray(x), dtype=None).astype(
        ml_dtypes.bfloat16)


def _f8(x):
    import ml_dtypes
    return np.ascontiguousarray(np.asarray(x)).astype(ml_dtypes.float8_e4m3)


def build_nc():
    nc = bacc.Bacc("TRN2", target_bir_lowering=False, debug=False,
                   num_devices=NC, num_swdge_queues=4)
    rg = [list(range(NC))]

    def inp(name, shape, dt=BF16):
        return nc.dram_tensor(name, list(shape), dt, kind="ExternalInput")

    io = dict(
        xT_bf=inp("xT_bf", (HID, T)),
        x_rows=inp("x_rows", (TSH, HID), F32),
        x_nat=inp("x_nat", (T, HID)),
        wqkv=inp("wqkv", (HID, 4 * D)),
        qnw=inp("qnw", (D, 1), F32),
        knw=inp("knw", (D, 1), F32),
        cosT=inp("cosT", (D, T), F32),
        sinT=inp("sinT", (D, T), F32),
        wo_r=inp("wo_r", (HPC * D, HID)),
        router=inp("router", (HID, E), F32),
        wsg=inp("wsg", (HID, SIS)),
        wsu=inp("wsu", (HID, SIS)),
        wsd=inp("wsd", (SIS, HID)),
        weg=inp("weg", (EPC, 128, KT, MI), FP8),
        weu=inp("weu", (EPC, 128, KT, MI), FP8),
        wed=inp("wed", (EPC, 128, MIT2, HID), FP8),
        ident=inp("ident", (128, 128)),
        identF=inp("identF", (128, 128), F32),
        trilS=inp("trilS", (128, 128), F32),
        ones128=inp("ones128", (128, 128), F32),
        onescol=inp("onescol", (128, 1), F32),
        onesrow=inp("onesrow", (1, 128), F32),
        iota32=inp("iota32", (128, E), F32),
        mcapbig=inp("mcapbig", (128, EPC), F32),
        iotaCE=inp("iotaCE", (128, EPC, CAP), F32),
        idcol=inp("idcol", (128, NT), F32),
        tril01=inp("tril01", (128, 128)),
        onescolb=inp("onescolb", (128, 1)),
        out_sh=nc.dram_tensor("out", [TSH, HID], F32, kind="ExternalOutput"),
        wo_part=nc.dram_tensor("wo_part", [T, HID], BF16),
        rs1=nc.dram_tensor("rs1", [TSH, HID], BF16),
        lg_sh=nc.dram_tensor("lg_sh", [TSH, E], F32),
        lg_all=nc.dram_tensor("lg_all", [T, E], F32, addr_space="Shared"),
        x2n_sh=nc.dram_tensor("x2n_sh", [TSH, HID], BF16),
        x2t_sh=nc.dram_tensor("x2t_sh", [HID, TSH], BF16),
        x2g=nc.dram_tensor("x2g", [T + 16, HID], BF16, addr_space="Shared"),
        x2t_all=nc.dram_tensor("x2t_all", [NC * HID, TSH], BF16,
                               addr_space="Shared"),
        ids_dram=nc.dram_tensor("ids_dram", [EPC * CAP, 1], F32),
        accum=nc.dram_tensor("accum", [T + 16, HID], BF16),
        rsf=nc.dram_tensor("rsf", [TSH, HID], BF16),
    )

    with tile.TileContext(nc) as tc:
        _build(tc, nc, rg, io)
    nc.compile()
    return nc


def _build(tc, nc, rg, io):
    g = lambda k: io[k]

    # =====================================================================
    # Phases 1-3: ln1 scales, QKV, attention, wo partial, ReduceScatter
    # =====================================================================
    with tc.tile_pool(name="const", bufs=1) as cpool:
        ident_sb = cpool.tile([128, 128], BF16, tag="ident")
        nc.sync.dma_start(ident_sb[:], g("ident").ap())
        cos_sb = cpool.tile([D, T], F32, tag="cos")
        nc.sync.dma_start(cos_sb[:], g("cosT").ap())
        sin_sb = cpool.tile([D, T], F32, tag="sin")
        nc.sync.dma_start(sin_sb[:], g("sinT").ap())
        qnw_sb = cpool.tile([D, 1], F32, tag="qnw")
        nc.sync.dma_start(qnw_sb[:], g("qnw").ap())
        knw_sb = cpool.tile([D, 1], F32, tag="knw")
        nc.sync.dma_start(knw_sb[:], g("knw").ap())
        onescol_sb = cpool.tile([128, 1], F32, tag="onescol")
        nc.sync.dma_start(onescol_sb[:], g("onescol").ap())
        onesrow_sb = cpool.tile([1, 128], F32, tag="onesrow")
        nc.sync.dma_start(onesrow_sb[:], g("onesrow").ap())
        tril01_sb = cpool.tile([128, 128], BF16, tag="tril01")
        nc.sync.dma_start(tril01_sb[:], g("tril01").ap())
        onescolb_sb = cpool.tile([128, 1], BF16, tag="onescolb")
        nc.sync.dma_start(onescolb_sb[:], g("onescolb").ap())
        identf_sb = cpool.tile([128, 128], F32, tag="identf")
        nc.sync.dma_start(identf_sb[:], g("identF").ap())
        eps_t = cpool.tile([128, 1], F32, tag="eps")
        nc.vector.memset(eps_t[:], EPS)
        epsD_t = cpool.tile([128, 1], F32, tag="epsD")
        nc.vector.memset(epsD_t[:], float(D) * EPS)

        # --- QKV + norms + rope + attention, pipelined per 512-chunk ---
        with tc.tile_pool(name="qk_f32", bufs=1) as qkp:
            qkT = [qkp.tile([128, T], F32, tag=f"qk{m}", name=f"qkT{m}")
                   for m in range(3)]
            vT = qkp.tile([128, T], F32, tag="vT")
            v_nat = qkp.tile([128, NT, D], BF16, tag="v_nat")
            s_nat = qkp.tile([128, NT], F32, tag="s_nat")
            rqk = [qkp.tile([1, T], F32, tag=f"rqk{m}", name=f"rqk{m}")
                   for m in range(3)]
            qk_bf = [qkp.tile([128, T], BF16, tag=f"rope{m}",
                              name=f"rope{m}")
                     for m in range(3)]
            attnT = [qkp.tile([128, T], BF16, tag=f"attnT{h}",
                                name=f"attnT{h}")
                     for h in range(HPC)]
            wo_sb = qkp.tile([128, HPC, HID], BF16, tag="wo_sb")
            nc.sync.dma_start(
                wo_sb[:],
                g("wo_r").ap().rearrange("(h p) m -> p h m", p=128))

            # ln1 scale s for ALL tokens, computed locally (no collective,
            # so it clears before attention chunk 0 needs v)
            # (loads ride the scalar queue so the sync queue leads with
            # QKV's x tiles and the weight prefetches)
            with tc.tile_pool(name="p1", bufs=2) as p1:
                for i in range(NT):
                    xr = p1.tile([128, HID], BF16, tag="xr")
                    nc.scalar.dma_start(xr[:],
                                        g("x_nat")[i * 128:(i + 1) * 128, :])
                    sq = p1.tile([128, HID], F32, tag="sq")
                    ssq = p1.tile([128, 1], F32, tag="ssq")
                    nc.scalar.activation(sq[:], xr[:], AF.Square,
                                         accum_out=ssq[:])
                    sr = p1.tile([128, 1], F32, tag="sr")
                    nc.scalar.activation(sr[:], ssq[:], AF.Sqrt,
                                         scale=1.0 / HID, bias=eps_t[:])
                    nc.vector.reciprocal(s_nat[:, i:i + 1], sr[:])

            # Transposed-score attention (see REV B notes): scores land as
            # [kv, t]; softmax runs without max subtraction; denominator via
            # ones-column matmuls into dp[h*32]; pss row-sums share dp's
            # bank at partition 64.  QKV, qk-norm, rope, v-transpose, the
            # attention j-loop, wo, and the chunked bf16 ReduceScatter all
            # ride one per-512-token-chunk loop so the tensor engine never
            # drains between phases.
            with tc.tile_pool(name="wqkvp", bufs=1) as wp, \
                 tc.tile_pool(name="xck", bufs=17) as xck, \
                 tc.tile_pool(name="nrm", bufs=2) as nrm, \
                 tc.tile_pool(name="att", bufs=3) as att, \
                 tc.tile_pool(name="awsb", bufs=3) as awsb, \
                 tc.tile_pool(name="qk2ps", bufs=1, space="PSUM") as qk2ps, \
                 tc.tile_pool(name="srow", bufs=2, space="PSUM") as srow, \
                 tc.tile_pool(name="apat", bufs=1, space="PSUM") as apat, \
                 tc.tile_pool(name="dpps", bufs=1, space="PSUM") as dpps, \
                 tc.tile_pool(name="awop", bufs=2, space="PSUM") as awop:
                wq_sb = wp.tile([128, KT, 4 * D], BF16)
                nc.sync.dma_start(
                    wq_sb[:],
                    g("wqkv").ap().rearrange("(k p) m -> p k m", p=128))
                for n in range(NG):
                    cg = slice(n * 512, (n + 1) * 512)
                    # --- QKV chunk: 4 m-passes over resident x tiles ---
                    xts = []
                    for k in range(KT):
                        xt = xck.tile([128, 512], BF16, tag="xt")
                        nc.sync.dma_start(
                            xt[:], g("xT_bf")[k * 128:(k + 1) * 128, cg])
                        xts.append(xt)
                    for m in range(4):
                        ps1 = qk2ps.tile([128, 512], F32, tag="ps1")
                        for k in range(KT):
                            nc.tensor.matmul(
                                ps1[:], wq_sb[:, k, m * 128:(m + 1) * 128],
                                xts[k][:], start=(k == 0), stop=(k == KT - 1))
                        dst = qkT[m] if m < 3 else vT
                        nc.vector.tensor_copy(dst[:, cg], ps1[:])
                    # --- qk-norm scales for this chunk ---
                    dp = dpps.tile([96, 512], F32, tag="dp")
                    for m in range(3):
                        sqc = nrm.tile([128, 512], F32, tag="sqc")
                        nc.vector.tensor_mul(sqc[:], qkT[m][:, cg],
                                             qkT[m][:, cg])
                        nc.tensor.matmul(dp[64:65, :], onescol_sb[:],
                                         sqc[:], start=True, stop=True,
                                         skip_group_check=True)
                        srt = nrm.tile([1, 512], F32, tag="srt")
                        if m < 2:
                            # q: D^-0.5 * rsqrt(mean+eps), softmax scale
                            # rides along
                            nc.scalar.activation(srt[:], dp[64:65, :],
                                                 AF.Sqrt,
                                                 bias=epsD_t[0:1, :])
                        else:
                            nc.scalar.activation(srt[:], dp[64:65, :],
                                                 AF.Sqrt, scale=1.0 / D,
                                                 bias=eps_t[0:1, :])
                        nc.vector.reciprocal(rqk[m][:, cg], srt[:])
                    # --- kn/qn + rope for this chunk ---
                    bck = awop.tile([128, 512], F32, tag="po")
                    nc.tensor.matmul(bck[:], onesrow_sb[:], rqk[2][:, cg],
                                     start=True, stop=True)
                    kn = nrm.tile([128, 512], F32, tag="kn")
                    nc.vector.scalar_tensor_tensor(
                        kn[:], qkT[2][:, cg], knw_sb[:], bck[:],
                        op0=OP.mult, op1=OP.mult)
                    srcs = []
                    for m in range(2):
                        qq = nrm.tile([128, 512], F32, tag=f"qn{m}")
                        nc.vector.tensor_scalar_mul(qq[:], qkT[m][:, cg],
                                                    qnw_sb[:])
                        srcs.append(qq)
                    srcs.append(kn)
                    for m in range(3):
                        qs = nrm.tile([128, 512], F32, tag="qs")
                        nc.scalar.copy(qs[0:HALF, :], srcs[m][HALF:D, :])
                        nc.scalar.copy(qs[HALF:D, :], srcs[m][0:HALF, :])
                        tt1 = nrm.tile([128, 512], F32, tag="tt1")
                        tt2 = nrm.tile([128, 512], F32, tag="tt2")
                        nc.vector.tensor_mul(tt1[:], srcs[m][:],
                                             cos_sb[:, cg])
                        nc.vector.tensor_mul(tt2[:], qs[:], sin_sb[:, cg])
                        nc.vector.tensor_add(qk_bf[m][:, cg], tt1[:],
                                             tt2[:])
                    # --- v -> natural layout, scaled by s ---
                    for jj in range(4):
                        j = 4 * n + jj
                        vv = srow.tile([128, 512], F32, tag="srw")
                        nc.tensor.transpose(
                            vv[:, 0:128], vT[:, j * 128:(j + 1) * 128],
                            identf_sb[:])
                        nc.vector.tensor_scalar_mul(v_nat[:, j, :],
                                                    vv[:, 0:128],
                                                    s_nat[:, j:j + 1])
                    # --- attention for this chunk ---
                    den_t = dp
                    pats = [apat.tile([128, 512], F32, tag=f"pat{h}",
                                      name=f"pat{h}_{n}")
                            for h in range(HPC)]
                    qsc = []
                    for h in range(HPC):
                        bcq = awop.tile([128, 512], F32, tag="po")
                        nc.tensor.matmul(
                            bcq[:], onesrow_sb[:],
                            rqk[h][:, n * 512:(n + 1) * 512],
                            start=True, stop=True)
                        qs = att.tile([128, 512], BF16, tag=f"qsc{h}")
                        nc.vector.tensor_mul(
                            qs[:], qk_bf[h][:, n * 512:(n + 1) * 512],
                            bcq[:])
                        qsc.append(qs)
                    jn = 4 * (n + 1)
                    for j in range(jn):
                        c0 = max(0, j * 128 - 512 * n)
                        for h in range(HPC):
                            srw = srow.tile([128, 512], F32, tag="srw")
                            nc.tensor.matmul(
                                srw[:, c0:512],
                                qk_bf[2][:, j * 128:(j + 1) * 128],
                                qsc[h][:, c0:512],
                                start=True, stop=True)
                            pT = att.tile([128, 512], BF16, tag=f"pT{h}")
                            nc.scalar.activation(pT[:, c0:512],
                                                 srw[:, c0:512], AF.Exp)
                            if j >= 4 * n:
                                nc.vector.tensor_mul(
                                    pT[:, c0:c0 + 128], pT[:, c0:c0 + 128],
                                    tril01_sb[:])
                            # NB: the PSUM pending-zero region of start=True
                            # covers only the OUT AP's partitions, so each
                            # head's 1-partition den row needs its own start.
                            nc.tensor.matmul(
                                den_t[h * 32:h * 32 + 1, c0:512],
                                onescolb_sb[:], pT[:, c0:512],
                                start=(j == 0),
                                stop=(j == jn - 1),
                                skip_group_check=True)
                            nc.tensor.matmul(
                                pats[h][:, c0:512], v_nat[:, j, :],
                                pT[:, c0:512],
                                start=(j == 0), stop=(j == jn - 1),
                                skip_group_check=True)
                    for h in range(HPC):
                        rden = att.tile([1, 512], F32, tag=f"rden{h}")
                        nc.vector.reciprocal(rden[:],
                                             den_t[h * 32:h * 32 + 1, :])
                        bcd = awop.tile([128, 512], F32, tag="po")
                        nc.tensor.matmul(bcd[:], onesrow_sb[:], rden[:],
                                         start=True, stop=True)
                        bcs = att.tile([128, 512], F32, tag="bcs")
                        nc.vector.tensor_copy(bcs[:], bcd[:])
                        nc.vector.tensor_mul(
                            attnT[h][:, n * 512:(n + 1) * 512],
                            pats[h][:], bcs[:])
                    # wo for this 512-token chunk, then its ReduceScatter
                    for tt in range(4):
                        t0 = (n * 4 + tt) * 128
                        for nn in range(4):
                            po = awop.tile([128, 512], F32, tag="po")
                            for h in range(HPC):
                                nc.tensor.matmul(
                                    po[:], attnT[h][:, t0:t0 + 128],
                                    wo_sb[:, h, nn * 512:(nn + 1) * 512],
                                    start=(h == 0), stop=(h == HPC - 1))
                            ob = awsb.tile([128, 512], BF16, tag="ob")
                            nc.vector.tensor_copy(ob[:], po[:])
                            nc.sync.dma_start(
                                g("wo_part")[t0:t0 + 128,
                                             nn * 512:(nn + 1) * 512], ob[:])
                    nc.gpsimd.collective_compute(
                        "ReduceScatter", OP.add, replica_groups=rg,
                        ins=[g("wo_part")[n * 512:(n + 1) * 512, :].opt()],
                        outs=[g("rs1")[n * 64:(n + 1) * 64, :].opt()])

    # =====================================================================
    # Phase 4: residual2, ln2, x2 (f32 + bf16), logits; AGs
    # =====================================================================
    with tc.tile_pool(name="keep", bufs=1) as keep:
        resid2 = keep.tile([128, TSH // 128, HID], F32, tag="resid2")
        ident2 = keep.tile([128, 128], F32, tag="ident2")
        nc.sync.dma_start(ident2[:], g("identF").ap())
        identb = keep.tile([128, 128], BF16, tag="identb")
        nc.sync.dma_start(identb[:], g("ident").ap())
        eps4_t = keep.tile([128, 1], F32, tag="eps4")
        nc.vector.memset(eps4_t[:], EPS)

        with tc.tile_pool(name="p4", bufs=2) as p4, \
             tc.tile_pool(name="p4f", bufs=1) as p4f, \
             tc.tile_pool(name="p4ps", bufs=4, space="PSUM") as p4ps, \
             tc.tile_pool(name="lgps", bufs=2, space="PSUM") as lgps:
            xt2f = p4f.tile([128, KT, TSH], F32, tag="xt2f")
            router_sb = p4f.tile([128, KT, E], F32, tag="router")
            nc.sync.dma_start(
                router_sb[:],
                g("router").ap().rearrange("(k p) e -> p k e", p=128))
            for i in range(TSH // 128):
                rsb = p4.tile([128, HID], BF16, tag="rsb")
                nc.scalar.dma_start(rsb[:],
                                    g("rs1")[i * 128:(i + 1) * 128, :])
                xr = p4.tile([128, HID], F32, tag="xr4")
                nc.sync.dma_start(xr[:], g("x_rows")[i * 128:(i + 1) * 128, :])
                nc.vector.tensor_add(resid2[:, i, :], rsb[:], xr[:])
                sq = p4.tile([128, HID], F32, tag="sq4")
                ssq = p4.tile([128, 1], F32, tag="ssq4")
                nc.scalar.activation(sq[:], resid2[:, i, :], AF.Square,
                                     accum_out=ssq[:])
                sr = p4.tile([128, 1], F32, tag="sr4")
                nc.scalar.activation(sr[:], ssq[:], AF.Sqrt, scale=1.0 / HID,
                                     bias=eps4_t[:])
                sv = p4.tile([128, 1], F32, tag="sv4")
                nc.vector.reciprocal(sv[:], sr[:])
                x2f = p4.tile([128, HID], F32, tag="x2f")
                nc.vector.tensor_scalar_mul(x2f[:], resid2[:, i, :], sv[:])
                # transposes + logits first so the logits AllGather (which
                # gates routing) is ready before the bulk x2 AllGathers
                for kh in range(KT):
                    pt = p4ps.tile([128, 128], F32, tag="pt4")
                    nc.tensor.transpose(
                        pt[:], x2f[:, kh * 128:(kh + 1) * 128], ident2[:])
                    nc.vector.tensor_copy(
                        xt2f[:, kh, i * 128:(i + 1) * 128], pt[:])
                lg = lgps.tile([128, E], F32, tag="lg")
                for kh in range(KT):
                    nc.tensor.matmul(
                        lg[:], xt2f[:, kh, i * 128:(i + 1) * 128],
                        router_sb[:, kh, :],
                        start=(kh == 0), stop=(kh == KT - 1))
                lgo = p4.tile([128, E], F32, tag="lgo")
                nc.vector.tensor_copy(lgo[:], lg[:])
                nc.sync.dma_start(g("lg_sh")[i * 128:(i + 1) * 128, :],
                                  lgo[:])
                x2b = p4.tile([128, HID], BF16, tag="x2b")
                nc.vector.tensor_copy(x2b[:], x2f[:])
                nc.sync.dma_start(g("x2n_sh")[i * 128:(i + 1) * 128, :],
                                  x2b[:])
                for kh in range(KT):
                    tb = p4.tile([128, 128], BF16, tag="tb4")
                    nc.vector.tensor_copy(
                        tb[:], xt2f[:, kh, i * 128:(i + 1) * 128])
                    nc.sync.dma_start(
                        g("x2t_sh")[kh * 128:(kh + 1) * 128,
                                    i * 128:(i + 1) * 128], tb[:])

            nc.gpsimd.collective_compute(
                "AllGather", OP.bypass, replica_groups=rg,
                ins=[g("lg_sh").ap().opt()], outs=[g("lg_all").ap().opt()])
            nc.gpsimd.collective_compute(
                "AllGather", OP.bypass, replica_groups=rg,
                ins=[g("x2n_sh").ap().opt()],
                outs=[g("x2g")[0:T, :].opt()])
            nc.gpsimd.collective_compute(
                "AllGather", OP.bypass, replica_groups=rg,
                ins=[g("x2t_sh").ap().opt()], outs=[g("x2t_all").ap().opt()])
            zz = p4.tile([16, HID], BF16, tag="zz")
            nc.vector.memset(zz[:], 0.0)
            nc.sync.dma_start(g("x2g")[T:T + 16, :], zz[:])

        # =================================================================
        # Phase 5: routing + dispatch tables (one-hot matmuls, no DGE)
        # =================================================================
        with tc.tile_pool(name="rtc", bufs=1) as rtc:
            iota_sb = rtc.tile([128, E], F32, tag="iota")
            nc.sync.dma_start(iota_sb[:], g("iota32").ap())
            mcap_sb = rtc.tile([128, EPC], F32, tag="mcap")
            nc.sync.dma_start(mcap_sb[:], g("mcapbig").ap())
            iotace_sb = rtc.tile([128, EPC, CAP], F32, tag="iotace")
            nc.sync.dma_start(iotace_sb[:], g("iotaCE").ap())
            idcol_sb = rtc.tile([128, NT], F32, tag="idcol")
            nc.sync.dma_start(idcol_sb[:], g("idcol").ap())
            tril_sb = rtc.tile([128, 128], F32, tag="tril")
            nc.sync.dma_start(tril_sb[:], g("trilS").ap())
            ones_sb = rtc.tile([128, 128], F32, tag="ones")
            nc.sync.dma_start(ones_sb[:], g("ones128").ap())

            w4_all = rtc.tile([128, NT, EPC], F32, tag="w4")
            msk = rtc.tile([128, NT, E], F32, tag="msk")
            pay = rtc.tile([128, NT, 1 + EPC], F32, tag="pay")
            idx16 = rtc.tile([128, EPC, CAP // 16], I16, tag="idx16")
            wsl_sb = rtc.tile([128, EPC, CAPT], F32, tag="wsl")
            ids_sb = rtc.tile([128, EPC, CAPT], F32, tag="idsb")

            with tc.tile_pool(name="rt", bufs=3) as rt, \
                 tc.tile_pool(name="rkps", bufs=2, space="PSUM") as rkps, \
                 tc.tile_pool(name="tabp", bufs=1, space="PSUM") as tabp:
                tabps = tabp.tile([128, EPC, CAPT, 1 + EPC], F32, tag="tab")
                for i in range(NT):
                    lgt = rt.tile([128, E], F32, tag="lgt")
                    nc.scalar.dma_start(lgt[:],
                                        g("lg_all")[i * 128:(i + 1) * 128, :])
                    gs = rt.tile([128, E], F32, tag="gs")
                    nc.scalar.activation(gs[:], lgt[:], AF.Sigmoid)
                    mx8 = rt.tile([128, 8], F32, tag="mx8")
                    ix8 = rt.tile([128, 8], U32, tag="ix8")
                    nc.vector.max_with_indices(mx8[:], ix8[:], gs[:])
                    sm = rt.tile([128, 1], F32, tag="sm")
                    nc.vector.tensor_reduce(sm[:], mx8[:, 0:K], axis=AX.X,
                                            op=OP.add)
                    rsm = rt.tile([128, 1], F32, tag="rsm")
                    nc.vector.reciprocal(rsm[:], sm[:])
                    nc.vector.tensor_scalar_mul(w4_all[:, i, :], mx8[:, 0:K],
                                                rsm[:])
                    ixf = rt.tile([128, K], F32, tag="ixf")
                    nc.vector.tensor_copy(ixf[:], ix8[:, 0:K])
                    comb = rt.tile([128, E], F32, tag="comb")
                    nc.vector.tensor_scalar(
                        comb[:], iota_sb[:], ixf[:, 0:1], w4_all[:, i, 0:1],
                        op0=OP.is_equal, op1=OP.mult)
                    oh = rt.tile([128, E], F32, tag="ohc")
                    for j in range(1, K):
                        nc.vector.tensor_scalar(
                            oh[:], iota_sb[:], ixf[:, j:j + 1],
                            w4_all[:, i, j:j + 1],
                            op0=OP.is_equal, op1=OP.mult)
                        nc.vector.tensor_add(comb[:], comb[:], oh[:])
                    nc.vector.tensor_scalar(msk[:, i, :], comb[:], 0.0, None,
                                            op0=OP.is_gt)
                    # dispatch payload: [token_id+1, combine weights e0..e3]
                    # The down-proj PSUM carries HSC*WSC*y (the gate-side WSC
                    # cancels inside the silu scale), so fold 1/(HSC*WSC)
                    # into the combine weights.
                    nc.vector.tensor_copy(pay[:, i, 0:1], idcol_sb[:, i:i + 1])
                    nc.vector.tensor_scalar(
                        pay[:, i, 1:1 + EPC], comb[:, 0:EPC],
                        1.0 / (HSC * WSC), None, op0=OP.mult)
                    # rank of token within each local expert (prefix count)
                    rk = rkps.tile([128, E], F32, tag="rk")
                    for s in range(i + 1):
                        nc.tensor.matmul(
                            rk[:], tril_sb[:] if s == i else ones_sb[:],
                            msk[:, s, :], start=(s == 0), stop=(s == i))
                    pen = rt.tile([128, EPC], F32, tag="pen")
                    nc.vector.tensor_scalar(
                        pen[:], msk[:, i, 0:EPC], -1.0e6, None, op0=OP.mult)
                    nc.vector.tensor_add(pen[:], pen[:], mcap_sb[:])
                    ovf = rt.tile([128, EPC], F32, tag="ovf")
                    nc.vector.tensor_scalar(
                        ovf[:], rk[:, 0:EPC], float(CAP) - 0.5, 1.0e6,
                        op0=OP.is_ge, op1=OP.mult)
                    offs = rt.tile([128, EPC], F32, tag="offs")
                    nc.vector.tensor_add(offs[:], pen[:], ovf[:])
                    nc.vector.tensor_add(offs[:], offs[:], rk[:, 0:EPC])
                    # one-hot of slot per expert; accumulate payload table
                    for e in range(EPC):
                        ohs = rt.tile([128, CAP], F32, tag="ohs")
                        nc.vector.tensor_scalar(
                            ohs[:], iotace_sb[:, e, :], offs[:, e:e + 1],
                            None, op0=OP.is_equal)
                        for sc in range(CAPT):
                            # All 12 (e, sc) groups live in one PSUM bank;
                            # start=True pending-zeroes the whole 2KB bank,
                            # so only the very first matmul may set it.
                            nc.tensor.matmul(
                                tabps[:, e, sc, :],
                                ohs[:, sc * 128:(sc + 1) * 128],
                                pay[:, i, :],
                                start=(i == 0 and e == 0 and sc == 0),
                                stop=(i == NT - 1),
                                skip_group_check=True)

                # unpack table: token ids -> DRAM (for idx16); weights -> SBUF
                for e in range(EPC):
                    for sc in range(CAPT):
                        nc.vector.tensor_copy(ids_sb[:, e, sc:sc + 1],
                                              tabps[:, e, sc, 0:1])
                        nc.vector.tensor_copy(wsl_sb[:, e, sc:sc + 1],
                                              tabps[:, e, sc, 1 + e:2 + e])
                nc.sync.dma_start(
                    g("ids_dram").ap().rearrange("(e s p) o -> p (e s o)",
                                                 p=128, s=CAPT),
                    ids_sb[:])
                # decode wrapped token lists (all experts at once):
                # id = slot - 1, empty slots (0) -> dummy row T
                idf = rt.tile([16, EPC * CAP // 16], F32, tag="idf")
                nc.scalar.dma_start(
                    idf[:],
                    g("ids_dram").ap().rearrange("(s p) o -> p (s o)", p=16))
                idm = rt.tile([16, EPC * CAP // 16], F32, tag="idm")
                nc.vector.tensor_scalar(
                    idm[:], idf[:], 0.5, float(T + 1),
                    op0=OP.is_lt, op1=OP.mult)
                nc.vector.tensor_add(idm[:], idm[:], idf[:])
                nc.vector.tensor_scalar(idm[:], idm[:], -1.0, None,
                                        op0=OP.add)
                idh = rt.tile([16, EPC * CAP // 16], I16, tag="idh")
                nc.vector.tensor_copy(idh[:], idm[:])
                for r in range(8):
                    nc.sync.dma_start(idx16[16 * r:16 * (r + 1), :, :],
                                      idh[:])

            # =============================================================
            # Phase 6: shared-expert MLP -> accum rows
            # =============================================================
            def x2t_chunk_ap(k, n):
                v = g("x2t_all").ap().rearrange("(c k p) s -> p k c s",
                                                p=128, k=KT)
                return v[:, k, 2 * n:2 * n + 2, :]

            with tc.tile_pool(name="shw", bufs=1) as shw, \
                 tc.tile_pool(name="shx", bufs=18) as shx, \
                 tc.tile_pool(name="shb", bufs=3) as shb, \
                 tc.tile_pool(name="shh", bufs=2) as shh, \
                 tc.tile_pool(name="shps", bufs=2, space="PSUM") as shps, \
                 tc.tile_pool(name="shdps", bufs=1, space="PSUM") as shdps:
                wsg_sb = shw.tile([128, KT, SIS], BF16, tag="wsg")
                nc.sync.dma_start(
                    wsg_sb[:],
                    g("wsg").ap().rearrange("(k p) m -> p k m", p=128))
                wsu_sb = shw.tile([128, KT, SIS], BF16, tag="wsu")
                nc.sync.dma_start(
                    wsu_sb[:],
                    g("wsu").ap().rearrange("(k p) m -> p k m", p=128))
                wsd_sb = shw.tile([128, 3, HID], BF16, tag="wsd")
                mdims = [128, 128, SIS - 256]
                for m in range(3):
                    nc.sync.dma_start(
                        wsd_sb[0:mdims[m], m, :],
                        g("wsd")[m * 128:m * 128 + mdims[m], :])
                for n in range(NG):
                    xts = []
                    for k in range(KT):
                        xt = shx.tile([128, 512], BF16, tag="shxt")
                        nc.scalar.dma_start(xt[:], x2t_chunk_ap(k, n))
                        xts.append(xt)
                    hsh = shh.tile([128, 3, 512], BF16, tag="hsh")
                    for m in range(3):
                        md = mdims[m]
                        pg = shps.tile([128, 512], F32, tag="pg")
                        pu = shps.tile([128, 512], F32, tag="pu")
                        for k in range(KT):
                            nc.tensor.matmul(
                                pg[0:md, :],
                                wsg_sb[:, k, m * 128:m * 128 + md],
                                xts[k][:], start=(k == 0),
                                stop=(k == KT - 1))
                            nc.tensor.matmul(
                                pu[0:md, :],
                                wsu_sb[:, k, m * 128:m * 128 + md],
                                xts[k][:], start=(k == 0),
                                stop=(k == KT - 1))
                        sg = shb.tile([128, 512], BF16, tag="sg")
                        nc.scalar.activation(sg[0:md, :], pg[0:md, :],
                                             AF.Silu)
                        nc.vector.tensor_mul(hsh[0:md, m, :], sg[0:md, :],
                                             pu[0:md, :])
                    for ts in range(4):
                        tok0 = n * 512 + ts * 128
                        pd4 = shdps.tile([128, 4, 512], F32, tag="pd4")
                        for m in range(3):
                            md = mdims[m]
                            for nh in range(4):
                                nc.tensor.matmul(
                                    pd4[:, nh, :],
                                    hsh[0:md, m, ts * 128:(ts + 1) * 128],
                                    wsd_sb[0:md, m, nh * 512:(nh + 1) * 512],
                                    start=(m == 0), stop=(m == 2))
                        for nh in range(4):
                            ob = shb.tile([128, 512], BF16, tag="shob")
                            nc.vector.tensor_copy(ob[:], pd4[:, nh, :])
                            nc.sync.dma_start(
                                g("accum")[tok0:tok0 + 128,
                                           nh * 512:(nh + 1) * 512], ob[:])

            # =============================================================
            # Phase 7: routed experts
            # =============================================================
            MIG = [(0, 4), (4, 8), (8, MIT)]      # mi groups for weights
            with tc.tile_pool(name="ew", bufs=18) as ewp, \
                 tc.tile_pool(name="edw", bufs=2) as edwp, \
                 tc.tile_pool(name="exn", bufs=2) as exn, \
                 tc.tile_pool(name="exf", bufs=1) as exf, \
                 tc.tile_pool(name="eh", bufs=2) as ehp, \
                 tc.tile_pool(name="ey", bufs=2) as eyp, \
                 tc.tile_pool(name="esc", bufs=3) as esc, \
                 tc.tile_pool(name="etp", bufs=2, space="PSUM") as etp, \
                 tc.tile_pool(name="eps", bufs=2, space="PSUM") as eps, \
                 tc.tile_pool(name="edps", bufs=2, space="PSUM") as edps:
                for e in range(EPC):
                    xet_nat = exn.tile([128, CAPT, HID], BF16, tag="xetn")
                    nc.gpsimd.dma_gather(
                        out_ap=xet_nat[:], in_ap=g("x2g").ap(),
                        idxs_ap=idx16[:, e, :], num_idxs=CAP,
                        num_idxs_reg=CAP, elem_size=HID, transpose=False,
                        queue_num=e % 2)
                    xet = exf.tile([128, KT, CAP], FP8, tag="xet")
                    for sc in range(CAPT):
                        for k in range(KT):
                            tp = etp.tile([128, 128], BF16, tag="tp")
                            nc.tensor.transpose(
                                tp[:], xet_nat[:, sc, k * 128:(k + 1) * 128],
                                identb[:])
                            nc.vector.tensor_copy(
                                xet[:, k, sc * 128:(sc + 1) * 128], tp[:])
                    # gate/up in fp8 DoubleRow: weights carry x WSC, silu
                    # rescales by 1/WSC; h stored fp8 at x HSC.
                    hsb = ehp.tile([128, MIT2, CAP], FP8, tag="hsb")
                    nc.vector.memset(hsb[:, MIT, :], 0.0)
                    for (m0, m1) in MIG:
                        mw = (m1 - m0) * 128
                        wgt, wut = [], []
                        for kp in range(KT // 2):
                            wg = ewp.tile([128, 2, 512], FP8, tag="wg")
                            nc.sync.dma_start(
                                wg[:, :, 0:mw],
                                g("weg")[e, :, 2 * kp:2 * kp + 2,
                                         m0 * 128:m1 * 128])
                            wu = ewp.tile([128, 2, 512], FP8, tag="wu")
                            nc.sync.dma_start(
                                wu[:, :, 0:mw],
                                g("weu")[e, :, 2 * kp:2 * kp + 2,
                                         m0 * 128:m1 * 128])
                            wgt.append(wg)
                            wut.append(wu)
                        for mi in range(m0, m1):
                            mo = (mi - m0) * 128
                            pg = eps.tile([128, CAP], F32, tag="epg")
                            pu = eps.tile([128, CAP], F32, tag="epu")
                            for kp in range(KT // 2):
                                nc.tensor.matmul(
                                    pg[:], wgt[kp][:, :, mo:mo + 128],
                                    xet[:, 2 * kp:2 * kp + 2, :],
                                    start=(kp == 0), stop=(kp == KT // 2 - 1),
                                    perf_mode=DR)
                                nc.tensor.matmul(
                                    pu[:], wut[kp][:, :, mo:mo + 128],
                                    xet[:, 2 * kp:2 * kp + 2, :],
                                    start=(kp == 0), stop=(kp == KT // 2 - 1),
                                    perf_mode=DR)
                            sg = esc.tile([128, CAP], BF16, tag="esg")
                            nc.scalar.activation(sg[:], pg[:], AF.Silu,
                                                 scale=1.0 / WSC)
                            nc.vector.scalar_tensor_tensor(
                                hsb[:, mi, :], pu[:], HSC / WSC, sg[:],
                                op0=OP.mult, op1=OP.mult)
                    ysb = eyp.tile([128, CAPT, HID], BF16, tag="ysb")
                    for hh in range(2):
                        wdt = edwp.tile([128, MIT2, 1024], FP8, tag="wd")
                        nc.sync.dma_start(
                            wdt[:],
                            g("wed")[e, :, :, hh * 1024:(hh + 1) * 1024])
                        for ct in range(CAPT):
                            pds = [edps.tile([128, 512], F32, tag="epd",
                                             name=f"pd{nh}")
                                   for nh in range(2)]
                            for mp in range(MIT2 // 2):
                                for nh in range(2):
                                    nc.tensor.matmul(
                                        pds[nh][:],
                                        hsb[:, 2 * mp:2 * mp + 2,
                                            ct * 128:(ct + 1) * 128],
                                        wdt[:, 2 * mp:2 * mp + 2,
                                            nh * 512:(nh + 1) * 512],
                                        start=(mp == 0),
                                        stop=(mp == MIT2 // 2 - 1),
                                        perf_mode=DR)
                            for nh in range(2):
                                nc.vector.tensor_scalar_mul(
                                    ysb[:, ct, hh * 1024 + nh * 512:
                                        hh * 1024 + (nh + 1) * 512],
                                    pds[nh][:], wsl_sb[:, e, ct:ct + 1])
                    nc.gpsimd.dma_scatter_add(
                        out_ap=g("accum").ap(), in_ap=ysb[:],
                        idxs_ap=idx16[:, e, :], num_idxs=CAP,
                        num_idxs_reg=CAP, elem_size=HID,
                        queue_num=2 + (e % 2))

        # =================================================================
        # Phase 8: final ReduceScatter + residual + output
        # =================================================================
        nc.gpsimd.collective_compute(
            "ReduceScatter", OP.add, replica_groups=rg,
            ins=[g("accum")[0:T, :].opt()], outs=[g("rsf").ap().opt()])
        with tc.tile_pool(name="p8", bufs=2) as p8:
            for i in range(TSH // 128):
                rb = p8.tile([128, HID], BF16, tag="rb8")
                nc.scalar.dma_start(rb[:], g("rsf")[i * 128:(i + 1) * 128, :])
                ov = p8.tile([128, HID], F32, tag="ov8")
                nc.vector.tensor_add(ov[:], rb[:], resid2[:, i, :])
                nc.sync.dma_start(g("out_sh")[i * 128:(i + 1) * 128, :],
                                  ov[:])


_nc_cache = None
TRACE = False          # set by test.py to capture exec_time_ns
LAST_RESULT = None


def prepare_in_maps_for_sim(inputs):
    """Host-side prep identical to kernel(); returns per-core in_maps."""
    return _prepare_in_maps({k: np.asarray(v) for k, v in inputs.items()})


def _get_nc():
    global _nc_cache
    if _nc_cache is None:
        _nc_cache = build_nc()
    return _nc_cache


def kernel(**inputs):
    inputs = {k: np.asarray(v) for k, v in inputs.items()}
    in_maps = _prepare_in_maps(inputs)
    nc = _get_nc()
    global LAST_RESULT
    res = run_bass_kernel_spmd(nc, in_maps, core_ids=list(range(NC)),
                               trace=TRACE)
    LAST_RESULT = res
    out = np.empty((T, HID), np.float32)
    for c in range(NC):
        out[_core_rows(c)] = res.results[c]["out"]
    return out.astype(np.float32)


def _core_rows(c):
    """Global token rows owned by core c (chunked-RS1 ownership)."""
    return np.concatenate(
        [np.arange(64) + n * 512 + c * 64 for n in range(NG)])


def _prepare_in_maps(inputs):
    pos = inputs["positions"].astype(np.float32)
    hs = inputs["hidden_states"].astype(np.float32)
    ln1_w = inputs["ln1_w"].astype(np.float32)
    ln2_w = inputs["ln2_w"].astype(np.float32)

    inv = 1.0 / (THETA ** (np.arange(HALF, dtype=np.float32) / HALF))
    ang = pos[None, :].astype(np.float64) * inv[:, None].astype(np.float64)
    cos_h = np.cos(ang).astype(np.float32)
    sin_h = np.sin(ang).astype(np.float32)
    cosT = np.vstack([cos_h, cos_h])
    sinT = np.vstack([-sin_h, sin_h])

    xT_bf = _bf(hs.T)
    x_nat_bf = _bf(hs)
    wq_f = ln1_w[:, None] * inputs["wq"].astype(np.float32)
    wk_f = ln1_w[:, None] * inputs["wk"].astype(np.float32)
    wv_f = ln1_w[:, None] * inputs["wv"].astype(np.float32)
    wo_f = inputs["wo"].astype(np.float32)
    router_f = ln2_w[:, None] * inputs["router_w"].astype(np.float32)
    wsg_f = ln2_w[:, None] * inputs["ws_gate"].astype(np.float32)
    wsu_f = ln2_w[:, None] * inputs["ws_up"].astype(np.float32)
    wsd_f = inputs["ws_down"].astype(np.float32)
    weg_f = ln2_w[None, :, None] * inputs["we_gate"].astype(np.float32)
    weu_f = ln2_w[None, :, None] * inputs["we_up"].astype(np.float32)
    wed_f = inputs["we_down"].astype(np.float32)

    WSC_np = np.float32(WSC)
    ident = np.eye(128, dtype=np.float32)
    trilS = np.triu(np.ones((128, 128), np.float32), 1)   # [k,m]=1 iff k<m
    ones128 = np.ones((128, 128), np.float32)
    onescol = np.ones((128, 1), np.float32)
    onesrow = np.ones((1, 128), np.float32)
    mcapbig = np.tile(
        (np.arange(EPC, dtype=np.float32) * CAP + 1.0e6)[None, :], (128, 1))
    iotaCE = np.tile(
        (np.arange(EPC, dtype=np.float32)[:, None] * CAP
         + np.arange(CAP, dtype=np.float32)[None, :])[None, :, :],
        (128, 1, 1))
    idcol = (np.arange(NT, dtype=np.float32)[None, :] * 128
             + np.arange(128, dtype=np.float32)[:, None] + 1.0)
    tril01 = (np.arange(128)[:, None] <= np.arange(128)[None, :]).astype(
        np.float32)

    in_maps = []
    for c in range(NC):
        kvh = (HPC * c) * KV // H
        perm = list(range(EPC * c, EPC * (c + 1))) + \
            [e for e in range(E) if not (EPC * c <= e < EPC * (c + 1))]
        iota32 = np.tile(np.asarray(perm, np.float32)[None, :], (128, 1))
        wqkv_c = np.concatenate([
            wq_f[:, (HPC * c) * D:(HPC * c + 2) * D],
            wk_f[:, kvh * D:(kvh + 1) * D],
            wv_f[:, kvh * D:(kvh + 1) * D],
        ], axis=1)
        m = {
            "xT_bf": xT_bf,
            "x_rows": np.ascontiguousarray(hs[_core_rows(c), :]),
            "x_nat": x_nat_bf,
            "wqkv": _bf(wqkv_c),
            "qnw": inputs["qnorm_w"].astype(np.float32).reshape(D, 1),
            "knw": inputs["knorm_w"].astype(np.float32).reshape(D, 1),
            "cosT": cosT, "sinT": sinT,
            "wo_r": _bf(wo_f[c * HPC * D:(c + 1) * HPC * D, :]),
            "router": np.ascontiguousarray(router_f),
            "wsg": _bf(wsg_f[:, c * SIS:(c + 1) * SIS]),
            "wsu": _bf(wsu_f[:, c * SIS:(c + 1) * SIS]),
            "wsd": _bf(wsd_f[c * SIS:(c + 1) * SIS, :]),
            "weg": _f8(
                (weg_f[EPC * c:EPC * (c + 1)] * WSC_np)
                .reshape(EPC, KT, 128, MI).transpose(0, 2, 1, 3)),
            "weu": _f8(
                (weu_f[EPC * c:EPC * (c + 1)] * WSC_np)
                .reshape(EPC, KT, 128, MI).transpose(0, 2, 1, 3)),
            "wed": _f8(np.concatenate([
                (wed_f[EPC * c:EPC * (c + 1)] * WSC_np)
                .reshape(EPC, MIT, 128, HID).transpose(0, 2, 1, 3),
                np.zeros((EPC, 128, MIT2 - MIT, HID), np.float32)], axis=2)),
            "ident": _bf(ident), "identF": ident,
            "trilS": trilS, "ones128": ones128,
            "onescol": onescol, "onesrow": onesrow,
            "iota32": iota32, "mcapbig": mcapbig,
            "iotaCE": iotaCE, "idcol": idcol,
            "tril01": _bf(tril01),
            "onescolb": _bf(onescol),
        }
        in_maps.append(m)
    return in_maps
